# revision 3
# baseline (speedup 1.0000x reference)
"""Causal self-attention (B=4, N=2048, D=1024, single head) on 8 TRN2 NeuronCores.

Sharding: core c handles batch b = c//2, query shard h = c%2 with the
stride-2 interleave q_global = 2*j + h  (j = 0..1023).  The interleave makes
the causal-mask *tile structure* identical on every core (SPMD-uniform), so
fully-masked score tiles can be skipped structurally while the residual
diagonal masking is handled with per-core data (query-position tensor).

Per-core pipeline (all matmuls bf16 inputs, f32 PSUM accumulation):
  QT[e,n]  = WqT_aug.T @ XTq_aug      (scale 1/sqrt(D) + bq folded into WqT_aug)
  KT[e,k]  = WkT_aug.T @ XT_aug       (bk folded)
  V[k,d]   = XT_aug.T @ WvT_aug       (bv folded)
  ST[k,j]  = KT.T @ QT                (scores, transposed layout)
  E        = exp(ST) * causal_mask    (no max-subtraction: |scores| <~ 2)
  rowsum[j]= ones.T @ E               (PE reduction over k partitions)
  CT[d,j]  = V.T @ E                  (unnormalized context)
  OT[e,j]  = WoT.T @ CT               (output proj)
  out      = OT * (1/rowsum) + bo     (normalization deferred to the end)

No collectives: each core receives exactly the host-side shard it needs.
"""

import os
import numpy as np
import ml_dtypes

BF16 = ml_dtypes.bfloat16

N_CORES = 8
B, N, D = 4, 2048, 1024
NQ = 1024           # queries per core
P = 128             # partitions
ET = D // P         # 8  e-tiles
CT_ = D // P        # 8  contraction tiles of D
KT_ALL = N // P     # 16 key tiles
JCW = 512           # query free-dim chunk
NJC = NQ // JCW     # 2

_cache = {}


def _build():
    from concourse import bacc, tile, mybir
    import concourse.bass as bass

    f32 = mybir.dt.float32
    bf16 = mybir.dt.bfloat16
    Exp = mybir.ActivationFunctionType.Exp
    is_ge = mybir.AluOpType.is_ge
    PSUM = bass.MemorySpace.PSUM

    nc = bacc.Bacc("TRN2", target_bir_lowering=False, debug=False,
                   num_devices=N_CORES)

    xt_d = nc.declare_dram_parameter("xt", [D, N], bf16, isOutput=False)
    xtq_d = nc.declare_dram_parameter("xtq", [D, NQ], bf16, isOutput=False)
    wqt_d = nc.declare_dram_parameter("wqt", [D + 1, D], bf16, isOutput=False)
    wkt_d = nc.declare_dram_parameter("wkt", [D + 1, D], bf16, isOutput=False)
    wvt_d = nc.declare_dram_parameter("wvt", [D + 1, D], bf16, isOutput=False)
    wot_d = nc.declare_dram_parameter("wot", [D, D], bf16, isOutput=False)
    bot_d = nc.declare_dram_parameter("bot", [P, ET], f32, isOutput=False)
    bqp_d = nc.declare_dram_parameter("bqpos", [P, NQ], f32, isOutput=False)
    kpt_d = nc.declare_dram_parameter("kpost", [P, KT_ALL], f32, isOutput=False)
    out_d = nc.declare_dram_parameter("out", [D, NQ], f32, isOutput=True)

    with tile.TileContext(nc) as tc:
        with (
            tc.tile_pool(name="consts", bufs=1) as p_c,
            tc.tile_pool(name="w", bufs=10) as p_w,
            tc.tile_pool(name="waug", bufs=3) as p_waug,
            tc.tile_pool(name="qt", bufs=ET) as p_qt,
            tc.tile_pool(name="kt", bufs=ET) as p_kt,
            tc.tile_pool(name="v", bufs=KT_ALL) as p_v,
            tc.tile_pool(name="ps", bufs=4, space=PSUM) as p_ps,
            tc.tile_pool(name="rsps", bufs=2, space=PSUM) as p_rs,
        ):
            ones_row = p_c.tile([1, JCW], bf16, tag="ones_row")
            nc.gpsimd.memset(ones_row[:], 1.0)
            ones_col = p_c.tile([P, 1], bf16, tag="ones_col")
            nc.gpsimd.memset(ones_col[:], 1.0)
            ones_col_f32 = p_c.tile([1, P], f32, tag="ones_col_f32")
            nc.gpsimd.memset(ones_col_f32[:], 1.0)
            bot_t = p_c.tile([P, ET], f32, tag="bot")
            nc.sync.dma_start(bot_t[:], bot_d[:, :])
            bqpos_t = p_c.tile([P, NQ], f32, tag="bqpos")
            nc.sync.dma_start(bqpos_t[:], bqp_d[:, :])
            kpost_t = p_c.tile([P, KT_ALL], f32, tag="kpost")
            nc.sync.dma_start(kpost_t[:], kpt_d[:, :])

            def load_w(dram, aug):
                ts = []
                for ct in range(CT_):
                    t = p_w.tile([P, D], bf16, tag="w", name="w")
                    nc.sync.dma_start(t[:], dram[ct * P:(ct + 1) * P, :])
                    ts.append(t)
                ta = None
                if aug:
                    ta = p_waug.tile([1, D], bf16, tag="waug", name="waug")
                    nc.sync.dma_start(ta[:], dram[D:D + 1, :])
                return ts, ta

            qt_tiles = [p_qt.tile([P, NQ], bf16, tag="qt", name="qt") for _ in range(ET)]
            kt_tiles = [p_kt.tile([P, N], bf16, tag="kt", name="kt") for _ in range(ET)]
            v_tiles = [p_v.tile([P, D], bf16, tag="v", name="v") for _ in range(KT_ALL)]

            with (
                tc.tile_pool(name="xt", bufs=CT_) as p_xt,
                tc.tile_pool(name="xtq", bufs=CT_) as p_xtq,
            ):
                # ---- Q projection ----
                wq, wqa = load_w(wqt_d, True)
                xtq_tiles = []
                for ct in range(CT_):
                    t = p_xtq.tile([P, NQ], bf16, tag="xtq", name="xtq")
                    nc.sync.dma_start(t[:], xtq_d[ct * P:(ct + 1) * P, :])
                    xtq_tiles.append(t)
                for et in range(ET):
                    for jc in range(NJC):
                        ps = p_ps.tile([P, JCW], f32, tag="ps", name="ps")
                        for ct in range(CT_):
                            nc.tensor.matmul(
                                ps[:],
                                wq[ct][:, et * P:(et + 1) * P],
                                xtq_tiles[ct][:, jc * JCW:(jc + 1) * JCW],
                                start=(ct == 0), stop=False)
                        nc.tensor.matmul(ps[:], wqa[:, et * P:(et + 1) * P],
                                         ones_row[:], start=False, stop=True)
                        nc.vector.tensor_copy(
                            qt_tiles[et][:, jc * JCW:(jc + 1) * JCW], ps[:])

                # ---- K projection ----
                wk, wka = load_w(wkt_d, True)
                xt_tiles = []
                for ct in range(CT_):
                    t = p_xt.tile([P, N], bf16, tag="xt", name="xt")
                    nc.sync.dma_start(t[:], xt_d[ct * P:(ct + 1) * P, :])
                    xt_tiles.append(t)
                for et in range(ET):
                    for kc in range(N // JCW):
                        ps = p_ps.tile([P, JCW], f32, tag="ps", name="ps")
                        for ct in range(CT_):
                            nc.tensor.matmul(
                                ps[:],
                                wk[ct][:, et * P:(et + 1) * P],
                                xt_tiles[ct][:, kc * JCW:(kc + 1) * JCW],
                                start=(ct == 0), stop=False)
                        nc.tensor.matmul(ps[:], wka[:, et * P:(et + 1) * P],
                                         ones_row[:], start=False, stop=True)
                        nc.scalar.copy(
                            kt_tiles[et][:, kc * JCW:(kc + 1) * JCW], ps[:])

                # ---- V projection ----
                wv, wva = load_w(wvt_d, True)
                for kt in range(KT_ALL):
                    for dc in range(D // JCW):
                        ps = p_ps.tile([P, JCW], f32, tag="ps", name="ps")
                        for ct in range(CT_):
                            nc.tensor.matmul(
                                ps[:],
                                xt_tiles[ct][:, kt * P:(kt + 1) * P],
                                wv[ct][:, dc * JCW:(dc + 1) * JCW],
                                start=(ct == 0), stop=False)
                        nc.tensor.matmul(ps[:], ones_row[:, 0:P],
                                         wva[:, dc * JCW:(dc + 1) * JCW],
                                         start=False, stop=True)
                        nc.scalar.copy(
                            v_tiles[kt][:, dc * JCW:(dc + 1) * JCW], ps[:])

            # Wo tiles (reuse the weight pool slots)
            wo, _ = load_w(wot_d, False)

            with (
                tc.tile_pool(name="exp", bufs=KT_ALL + 1) as p_exp,
                tc.tile_pool(name="raw", bufs=2) as p_raw,
                tc.tile_pool(name="mask", bufs=2) as p_mask,
                tc.tile_pool(name="ctx", bufs=ET) as p_ctx,
                tc.tile_pool(name="of", bufs=4) as p_of,
                tc.tile_pool(name="brec", bufs=2) as p_brec,
                tc.tile_pool(name="recip", bufs=2) as p_recip,
            ):
                for jc in range(NJC):
                    jsl = slice(jc * JCW, (jc + 1) * JCW)
                    nkt = 8 if jc == 0 else 16   # causal: skip fully-masked
                    rs_ps = p_rs.tile([1, JCW], f32, tag="rsps", name="rsps")
                    exps = []
                    for kt in range(nkt):
                        st = p_ps.tile([P, JCW], f32, tag="ps", name="ps")
                        for et in range(ET):
                            nc.tensor.matmul(
                                st[:],
                                kt_tiles[et][:, kt * P:(kt + 1) * P],
                                qt_tiles[et][:, jsl],
                                start=(et == 0), stop=(et == ET - 1))
                        ex = p_exp.tile([P, JCW], bf16, tag="exp", name="exp")
                        boundary = (kt >= 8 * jc)
                        if boundary:
                            raw = p_raw.tile([P, JCW], bf16, tag="raw", name="raw")
                            nc.scalar.activation(raw[:], st[:], Exp)
                            msk = p_mask.tile([P, JCW], bf16, tag="mask", name="mask")
                            nc.vector.tensor_scalar(
                                msk[:], bqpos_t[:, jsl],
                                kpost_t[:, kt:kt + 1], None, is_ge)
                            nc.vector.tensor_mul(ex[:], raw[:], msk[:])
                        else:
                            nc.scalar.activation(ex[:], st[:], Exp)
                        exps.append(ex)
                        nc.tensor.matmul(rs_ps[:], ones_col[:], ex[:],
                                         start=(kt == 0), stop=(kt == nkt - 1))

                    recip_t = p_recip.tile([1, JCW], f32, tag="recip", name="recip")
                    nc.vector.reciprocal(recip_t[:], rs_ps[:])
                    br_ps = p_ps.tile([P, JCW], f32, tag="ps", name="ps")
                    nc.tensor.matmul(br_ps[:], ones_col_f32[:], recip_t[:],
                                     start=True, stop=True)
                    brec_t = p_brec.tile([P, JCW], f32, tag="brec", name="brec")
                    nc.vector.tensor_copy(brec_t[:], br_ps[:])

                    ctxs = []
                    for dt in range(ET):
                        cps = p_ps.tile([P, JCW], f32, tag="ps", name="ps")
                        for kt in range(nkt):
                            nc.tensor.matmul(
                                cps[:],
                                v_tiles[kt][:, dt * P:(dt + 1) * P],
                                exps[kt][:],
                                start=(kt == 0), stop=(kt == nkt - 1))
                        ct_t = p_ctx.tile([P, JCW], bf16, tag="ctx", name="ctx")
                        nc.scalar.copy(ct_t[:], cps[:])
                        ctxs.append(ct_t)

                    for et in range(ET):
                        ops_ = p_ps.tile([P, JCW], f32, tag="ps", name="ps")
                        for dt in range(ET):
                            nc.tensor.matmul(
                                ops_[:],
                                wo[dt][:, et * P:(et + 1) * P],
                                ctxs[dt][:],
                                start=(dt == 0), stop=(dt == ET - 1))
                        of1 = p_of.tile([P, JCW], f32, tag="of", name="of")
                        nc.vector.tensor_mul(of1[:], ops_[:], brec_t[:])
                        of2 = p_of.tile([P, JCW], f32, tag="of", name="of")
                        nc.vector.tensor_scalar_add(of2[:], of1[:],
                                                    bot_t[:, et:et + 1])
                        nc.sync.dma_start(out_d[et * P:(et + 1) * P, jsl],
                                          of2[:])

    nc.compile()
    return nc


def _prep_in_maps(X, Wq, bq, Wk, bk, Wv, bv, Wo, bo):
    s = np.float32(1.0 / np.sqrt(np.float32(D)))
    wqt = (np.concatenate([Wq.T, bq[None, :]], axis=0) * s).astype(BF16)
    wkt = np.concatenate([Wk.T, bk[None, :]], axis=0).astype(BF16)
    wvt = np.concatenate([Wv.T, bv[None, :]], axis=0).astype(BF16)
    wot = np.ascontiguousarray(Wo.T).astype(BF16)
    bot = np.ascontiguousarray(bo.reshape(ET, P).T).astype(np.float32)
    kpost = np.ascontiguousarray(
        np.arange(N, dtype=np.float32).reshape(KT_ALL, P).T)

    in_maps = []
    for c in range(N_CORES):
        b, h = c // 2, c % 2
        Xb = X[b]
        xt = np.ascontiguousarray(Xb.T).astype(BF16)
        xtq = np.ascontiguousarray(Xb[h::2].T).astype(BF16)
        qpos = (2.0 * np.arange(NQ, dtype=np.float32) + h)
        bqpos = np.ascontiguousarray(
            np.broadcast_to(qpos[None, :], (P, NQ))).astype(np.float32)
        in_maps.append({
            "xt": xt, "xtq": xtq,
            "wqt": wqt, "wkt": wkt, "wvt": wvt, "wot": wot,
            "bot": bot, "bqpos": bqpos, "kpost": kpost,
        })
    return in_maps


last_exec_time_ns = None


def _ensure_ntff_hook():
    """Register the axon NTFF profile hook if the image's antenv lacks it."""
    try:
        from antenv.axon_hooks import get_axon_ntff_profile_hook  # noqa: F401
        return
    except ImportError:
        pass
    import sys
    import types
    mod = types.ModuleType("antenv.axon_hooks")
    mod._hook = None
    mod.set_axon_ntff_profile_hook = lambda h: setattr(mod, "_hook", h)
    mod.get_axon_ntff_profile_hook = lambda: mod._hook
    sys.modules["antenv.axon_hooks"] = mod
    try:
        import antenv
        antenv.axon_hooks = mod
    except ImportError:
        pass
    try:
        from trn_agent_boot.trn_boot import _ntff_profile_via_ctypes
        mod._hook = _ntff_profile_via_ctypes("/opt/axon/libaxon_pjrt.so")
    except Exception:
        pass


def kernel(X, Wq, bq, Wk, bk, Wv, bv, Wo, bo):
    global last_exec_time_ns
    from concourse.bass_utils import run_bass_kernel_spmd
    _ensure_ntff_hook()

    X = np.asarray(X, dtype=np.float32)
    args = [np.asarray(a, dtype=np.float32)
            for a in (Wq, bq, Wk, bk, Wv, bv, Wo, bo)]

    if "nc" not in _cache:
        _cache["nc"] = _build()
    nc = _cache["nc"]

    in_maps = _prep_in_maps(X, *args)
    kwargs = {}
    tmpdir = os.environ.get("KERNEL_TRACE_DIR")
    if tmpdir:
        kwargs = dict(trace=True, tmpdir=tmpdir)
    res = run_bass_kernel_spmd(nc, in_maps, core_ids=list(range(N_CORES)),
                               **kwargs)
    last_exec_time_ns = res.exec_time_ns

    out = np.empty((B, N, D), dtype=np.float32)
    for c in range(N_CORES):
        b, h = c // 2, c % 2
        out[b, h::2, :] = np.asarray(res.results[c]["out"],
                                     dtype=np.float32).T
    return out


# revision 4
# speedup vs baseline: 1.1051x; 1.1051x over previous
"""Causal self-attention (B=4, N=2048, D=1024, single head) on 8 TRN2 NeuronCores.

Sharding: core c handles batch b = c//2, query shard h = c%2 with the
stride-2 interleave q_global = 2*j + h  (j = 0..1023).  The interleave makes
the causal-mask *tile structure* identical on every core (SPMD-uniform), so
fully-masked score tiles can be skipped structurally while the residual
diagonal masking is handled with per-core data (query-position tensor).

Per-core pipeline (all matmuls bf16 inputs, f32 PSUM accumulation):
  QT[e,n]  = WqT.T @ XTq   (+bq/32 folded into the PSUM->SBUF eviction)
  KT[e,k]  = WkT.T @ XT    (+bk in eviction)
  V[k,d]   = XT.T @ WvT    (+bv broadcast tile in eviction)
  ST[k,j]  = KT.T @ QT     (scores; 1/sqrt(D) folded into WqT host-side)
  E        = exp(ST) * causal_mask    (no max-subtraction: |scores| <~ 2)
  rowsum[j]= ones.T @ E    (PE reduction over k partitions)
  CT[d,j]  = V.T @ E
  OT[e,j]  = WoT.T @ CT
  out      = OT * (1/rowsum) + bo     (normalization deferred to the end)

Loops are ordered so each stationary (lhsT) operand feeds several
back-to-back matmuls, and PSUM evictions all run on the Vector engine.
No collectives: each core receives exactly the host-side shard it needs.
"""

import os
import numpy as np
import ml_dtypes

BF16 = ml_dtypes.bfloat16

N_CORES = 8
B, N, D = 4, 2048, 1024
NQ = 1024           # queries per core
P = 128             # partitions
ET = D // P         # 8  e-tiles
CT_ = D // P        # 8  contraction tiles of D
KT_ALL = N // P     # 16 key tiles
JCW = 512           # free-dim chunk
NJC = NQ // JCW     # 2

_cache = {}


def _build():
    from concourse import bacc, tile, mybir
    import concourse.bass as bass

    f32 = mybir.dt.float32
    bf16 = mybir.dt.bfloat16
    Exp = mybir.ActivationFunctionType.Exp
    is_ge = mybir.AluOpType.is_ge
    add = mybir.AluOpType.add
    PSUM = bass.MemorySpace.PSUM

    nc = bacc.Bacc("TRN2", target_bir_lowering=False, debug=False,
                   num_devices=N_CORES)

    xt_d = nc.declare_dram_parameter("xt", [D, N], bf16, isOutput=False)
    xtq_d = nc.declare_dram_parameter("xtq", [D, NQ], bf16, isOutput=False)
    wqt_d = nc.declare_dram_parameter("wqt", [D, D], bf16, isOutput=False)
    wkt_d = nc.declare_dram_parameter("wkt", [D, D], bf16, isOutput=False)
    wvt_d = nc.declare_dram_parameter("wvt", [D, D], bf16, isOutput=False)
    wot_d = nc.declare_dram_parameter("wot", [D, D], bf16, isOutput=False)
    bqt_d = nc.declare_dram_parameter("bqt", [P, ET], f32, isOutput=False)
    bkt_d = nc.declare_dram_parameter("bkt", [P, ET], f32, isOutput=False)
    bbv_d = nc.declare_dram_parameter("bbv", [P, D], f32, isOutput=False)
    bot_d = nc.declare_dram_parameter("bot", [P, ET], f32, isOutput=False)
    bqp_d = nc.declare_dram_parameter("bqpos", [P, NQ], f32, isOutput=False)
    kpt_d = nc.declare_dram_parameter("kpost", [P, KT_ALL], f32, isOutput=False)
    out_d = nc.declare_dram_parameter("out", [D, NQ], f32, isOutput=True)

    with tile.TileContext(nc) as tc:
        with (
            tc.tile_pool(name="consts", bufs=1) as p_c,
            tc.tile_pool(name="w", bufs=10) as p_w,
            tc.tile_pool(name="qt", bufs=ET) as p_qt,
            tc.tile_pool(name="kt", bufs=ET) as p_kt,
            tc.tile_pool(name="v", bufs=KT_ALL) as p_v,
            tc.tile_pool(name="ps", bufs=5, space=PSUM) as p_ps,
            tc.tile_pool(name="rsps", bufs=2, space=PSUM) as p_rs,
        ):
            qt_tiles = [p_qt.tile([P, NQ], bf16, tag="qt", name="qt")
                        for _ in range(ET)]
            kt_tiles = [p_kt.tile([P, N], bf16, tag="kt", name="kt")
                        for _ in range(ET)]
            v_tiles = [p_v.tile([P, D], bf16, tag="v", name="v")
                       for _ in range(KT_ALL)]

            def load_w(dram):
                ts = []
                for ct in range(CT_):
                    t = p_w.tile([P, D], bf16, tag="w", name="w")
                    nc.sync.dma_start(t[:], dram[ct * P:(ct + 1) * P, :])
                    ts.append(t)
                return ts

            with (
                tc.tile_pool(name="xt", bufs=CT_) as p_xt,
                tc.tile_pool(name="xtq", bufs=CT_) as p_xtq,
            ):
                # ---- Q projection (DMAs interleaved: weight tile then X tile
                # so the PE can start on the first accumulation group ASAP) ----
                wq = []
                xtq_tiles = []
                for ct in range(CT_):
                    t = p_w.tile([P, D], bf16, tag="w", name="w")
                    nc.sync.dma_start(t[:], wqt_d[ct * P:(ct + 1) * P, :])
                    wq.append(t)
                    t2 = p_xtq.tile([P, NQ], bf16, tag="xtq", name="xtq")
                    nc.gpsimd.dma_start(t2[:], xtq_d[ct * P:(ct + 1) * P, :])
                    xtq_tiles.append(t2)
                bqt_t = p_c.tile([P, ET], f32, tag="bqt")
                nc.scalar.dma_start(bqt_t[:], bqt_d[:, :])

                for et in range(ET):
                    pss = [p_ps.tile([P, JCW], f32, tag="ps", name="ps")
                           for _ in range(NJC)]
                    for ct in range(CT_):
                        for jc in range(NJC):
                            nc.tensor.matmul(
                                pss[jc][:],
                                wq[ct][:, et * P:(et + 1) * P],
                                xtq_tiles[ct][:, jc * JCW:(jc + 1) * JCW],
                                start=(ct == 0), stop=(ct == CT_ - 1))
                    for jc in range(NJC):
                        nc.vector.tensor_scalar_add(
                            qt_tiles[et][:, jc * JCW:(jc + 1) * JCW],
                            pss[jc][:], bqt_t[:, et:et + 1])

                # ---- K projection ----
                wk = []
                xt_tiles = []
                for ct in range(CT_):
                    t = p_w.tile([P, D], bf16, tag="w", name="w")
                    nc.sync.dma_start(t[:], wkt_d[ct * P:(ct + 1) * P, :])
                    wk.append(t)
                    t2 = p_xt.tile([P, N], bf16, tag="xt", name="xt")
                    nc.gpsimd.dma_start(t2[:], xt_d[ct * P:(ct + 1) * P, :])
                    xt_tiles.append(t2)
                bkt_t = p_c.tile([P, ET], f32, tag="bkt")
                nc.scalar.dma_start(bkt_t[:], bkt_d[:, :])

                for et in range(ET):
                    for kh in range(2):          # halves of the 4 k-chunks
                        pss = [p_ps.tile([P, JCW], f32, tag="ps", name="ps")
                               for _ in range(2)]
                        for ct in range(CT_):
                            for i, kc in enumerate((2 * kh, 2 * kh + 1)):
                                nc.tensor.matmul(
                                    pss[i][:],
                                    wk[ct][:, et * P:(et + 1) * P],
                                    xt_tiles[ct][:, kc * JCW:(kc + 1) * JCW],
                                    start=(ct == 0), stop=(ct == CT_ - 1))
                        for i, kc in enumerate((2 * kh, 2 * kh + 1)):
                            nc.vector.tensor_scalar_add(
                                kt_tiles[et][:, kc * JCW:(kc + 1) * JCW],
                                pss[i][:], bkt_t[:, et:et + 1])

                # ---- V projection ----
                wv = load_w(wvt_d)
                bbv_t = p_c.tile([P, D], f32, tag="bbv")
                nc.scalar.dma_start(bbv_t[:], bbv_d[:, :])
                for kt in range(KT_ALL):
                    pss = [p_ps.tile([P, JCW], f32, tag="ps", name="ps")
                           for _ in range(2)]
                    for ct in range(CT_):
                        for dc in range(2):
                            nc.tensor.matmul(
                                pss[dc][:],
                                xt_tiles[ct][:, kt * P:(kt + 1) * P],
                                wv[ct][:, dc * JCW:(dc + 1) * JCW],
                                start=(ct == 0), stop=(ct == CT_ - 1))
                    for dc in range(2):
                        nc.vector.tensor_tensor(
                            v_tiles[kt][:, dc * JCW:(dc + 1) * JCW],
                            pss[dc][:], bbv_t[:, dc * JCW:(dc + 1) * JCW], add)

            # Wo tiles + remaining consts
            wo = load_w(wot_d)
            ones_col = p_c.tile([P, 1], bf16, tag="ones_col")
            nc.gpsimd.memset(ones_col[:], 1.0)
            ones_col_f32 = p_c.tile([1, P], f32, tag="ones_col_f32")
            nc.gpsimd.memset(ones_col_f32[:], 1.0)
            bot_t = p_c.tile([P, ET], f32, tag="bot")
            nc.scalar.dma_start(bot_t[:], bot_d[:, :])
            bqpos_t = p_c.tile([P, NQ], f32, tag="bqpos")
            nc.scalar.dma_start(bqpos_t[:], bqp_d[:, :])
            kpost_t = p_c.tile([P, KT_ALL], f32, tag="kpost")
            nc.scalar.dma_start(kpost_t[:], kpt_d[:, :])

            with (
                tc.tile_pool(name="exp", bufs=KT_ALL + ET + 1) as p_exp,
                tc.tile_pool(name="raw", bufs=2) as p_raw,
                tc.tile_pool(name="mask", bufs=2) as p_mask,
                tc.tile_pool(name="ctx", bufs=2 * ET + 1) as p_ctx,
                tc.tile_pool(name="of", bufs=4) as p_of,
                tc.tile_pool(name="brec", bufs=2) as p_brec,
                tc.tile_pool(name="recip", bufs=2) as p_recip,
            ):
                # jc=0 covers global queries [0,1024): keys < 1024 (kt 0..7).
                # jc=1 covers [1024,2048): all 16 kt; kt 0..7 unmasked there.
                def jcs_of(kt):
                    return (0, 1) if kt < 8 else (1,)

                # ---- scores + exp + mask + rowsum ----
                rs_ps = {jc: p_rs.tile([1, JCW], f32, tag="rsps", name="rsps")
                         for jc in range(NJC)}
                exps = {}
                for kt in range(KT_ALL):
                    sts = {}
                    for jc in jcs_of(kt):
                        sts[jc] = p_ps.tile([P, JCW], f32, tag="ps", name="ps")
                    for et in range(ET):
                        for jc in jcs_of(kt):
                            nc.tensor.matmul(
                                sts[jc][:],
                                kt_tiles[et][:, kt * P:(kt + 1) * P],
                                qt_tiles[et][:, jc * JCW:(jc + 1) * JCW],
                                start=(et == 0), stop=(et == ET - 1))
                    for jc in jcs_of(kt):
                        ex = p_exp.tile([P, JCW], bf16, tag="exp", name="exp")
                        boundary = (kt >= 8 * jc)
                        if boundary:
                            raw = p_raw.tile([P, JCW], bf16, tag="raw",
                                             name="raw")
                            nc.scalar.activation(raw[:], sts[jc][:], Exp)
                            msk = p_mask.tile([P, JCW], bf16, tag="mask",
                                              name="mask")
                            nc.vector.tensor_scalar(
                                msk[:],
                                bqpos_t[:, jc * JCW:(jc + 1) * JCW],
                                kpost_t[:, kt:kt + 1], None, is_ge)
                            nc.vector.tensor_mul(ex[:], raw[:], msk[:])
                        else:
                            nc.scalar.activation(ex[:], sts[jc][:], Exp)
                        exps[(jc, kt)] = ex
                        nkt = 8 if jc == 0 else 16
                        nc.tensor.matmul(rs_ps[jc][:], ones_col[:], ex[:],
                                         start=(kt == 0), stop=(kt == nkt - 1))

                # ---- 1/rowsum broadcast tiles ----
                brec = {}
                for jc in range(NJC):
                    recip_t = p_recip.tile([1, JCW], f32, tag="recip",
                                           name="recip")
                    nc.vector.reciprocal(recip_t[:], rs_ps[jc][:])
                    br_ps = p_ps.tile([P, JCW], f32, tag="ps", name="ps")
                    nc.tensor.matmul(br_ps[:], ones_col_f32[:], recip_t[:],
                                     start=True, stop=True)
                    bt = p_brec.tile([P, JCW], f32, tag="brec", name="brec")
                    nc.vector.tensor_copy(bt[:], br_ps[:])
                    brec[jc] = bt

                # ---- context ----
                ctxs = {}
                for dt in range(ET):
                    cps = {jc: p_ps.tile([P, JCW], f32, tag="ps", name="ps")
                           for jc in range(NJC)}
                    for kt in range(KT_ALL):
                        for jc in jcs_of(kt):
                            nkt = 8 if jc == 0 else 16
                            nc.tensor.matmul(
                                cps[jc][:],
                                v_tiles[kt][:, dt * P:(dt + 1) * P],
                                exps[(jc, kt)][:],
                                start=(kt == 0), stop=(kt == nkt - 1))
                    for jc in range(NJC):
                        ct_t = p_ctx.tile([P, JCW], bf16, tag="ctx",
                                          name="ctx")
                        nc.vector.tensor_copy(ct_t[:], cps[jc][:])
                        ctxs[(jc, dt)] = ct_t

                # ---- output projection + normalize + bias ----
                for et in range(ET):
                    opss = {jc: p_ps.tile([P, JCW], f32, tag="ps", name="ps")
                            for jc in range(NJC)}
                    for dt in range(ET):
                        for jc in range(NJC):
                            nc.tensor.matmul(
                                opss[jc][:],
                                wo[dt][:, et * P:(et + 1) * P],
                                ctxs[(jc, dt)][:],
                                start=(dt == 0), stop=(dt == ET - 1))
                    for jc in range(NJC):
                        jsl = slice(jc * JCW, (jc + 1) * JCW)
                        of1 = p_of.tile([P, JCW], f32, tag="of", name="of")
                        nc.vector.tensor_mul(of1[:], opss[jc][:], brec[jc][:])
                        of2 = p_of.tile([P, JCW], f32, tag="of", name="of")
                        nc.vector.tensor_scalar_add(of2[:], of1[:],
                                                    bot_t[:, et:et + 1])
                        nc.sync.dma_start(out_d[et * P:(et + 1) * P, jsl],
                                          of2[:])

    nc.compile()
    return nc


def _prep_in_maps(X, Wq, bq, Wk, bk, Wv, bv, Wo, bo):
    s = np.float32(1.0 / np.sqrt(np.float32(D)))
    wqt = np.ascontiguousarray(Wq.T * s).astype(BF16)
    wkt = np.ascontiguousarray(Wk.T).astype(BF16)
    wvt = np.ascontiguousarray(Wv.T).astype(BF16)
    wot = np.ascontiguousarray(Wo.T).astype(BF16)
    bqt = np.ascontiguousarray((bq * s).reshape(ET, P).T).astype(np.float32)
    bkt = np.ascontiguousarray(bk.reshape(ET, P).T).astype(np.float32)
    bbv = np.ascontiguousarray(
        np.broadcast_to(bv[None, :], (P, D))).astype(np.float32)
    bot = np.ascontiguousarray(bo.reshape(ET, P).T).astype(np.float32)
    kpost = np.ascontiguousarray(
        np.arange(N, dtype=np.float32).reshape(KT_ALL, P).T)

    in_maps = []
    for c in range(N_CORES):
        b, h = c // 2, c % 2
        Xb = X[b]
        xt = np.ascontiguousarray(Xb.T).astype(BF16)
        xtq = np.ascontiguousarray(Xb[h::2].T).astype(BF16)
        qpos = (2.0 * np.arange(NQ, dtype=np.float32) + h)
        bqpos = np.ascontiguousarray(
            np.broadcast_to(qpos[None, :], (P, NQ))).astype(np.float32)
        in_maps.append({
            "xt": xt, "xtq": xtq,
            "wqt": wqt, "wkt": wkt, "wvt": wvt, "wot": wot,
            "bqt": bqt, "bkt": bkt, "bbv": bbv, "bot": bot,
            "bqpos": bqpos, "kpost": kpost,
        })
    return in_maps


last_exec_time_ns = None


def _ensure_ntff_hook():
    """Register the axon NTFF profile hook if the image's antenv lacks it."""
    try:
        from antenv.axon_hooks import get_axon_ntff_profile_hook  # noqa: F401
        return
    except ImportError:
        pass
    import sys
    import types
    mod = types.ModuleType("antenv.axon_hooks")
    mod._hook = None
    mod.set_axon_ntff_profile_hook = lambda h: setattr(mod, "_hook", h)
    mod.get_axon_ntff_profile_hook = lambda: mod._hook
    sys.modules["antenv.axon_hooks"] = mod
    try:
        import antenv
        antenv.axon_hooks = mod
    except ImportError:
        pass
    try:
        from trn_agent_boot.trn_boot import _ntff_profile_via_ctypes
        mod._hook = _ntff_profile_via_ctypes("/opt/axon/libaxon_pjrt.so")
    except Exception:
        pass


def kernel(X, Wq, bq, Wk, bk, Wv, bv, Wo, bo):
    global last_exec_time_ns
    from concourse.bass_utils import run_bass_kernel_spmd
    _ensure_ntff_hook()

    X = np.asarray(X, dtype=np.float32)
    args = [np.asarray(a, dtype=np.float32)
            for a in (Wq, bq, Wk, bk, Wv, bv, Wo, bo)]

    if "nc" not in _cache:
        _cache["nc"] = _build()
    nc = _cache["nc"]

    in_maps = _prep_in_maps(X, *args)
    kwargs = {}
    tmpdir = os.environ.get("KERNEL_TRACE_DIR")
    if tmpdir:
        kwargs = dict(trace=True, tmpdir=tmpdir)
    res = run_bass_kernel_spmd(nc, in_maps, core_ids=list(range(N_CORES)),
                               **kwargs)
    last_exec_time_ns = res.exec_time_ns

    out = np.empty((B, N, D), dtype=np.float32)
    for c in range(N_CORES):
        b, h = c // 2, c % 2
        out[b, h::2, :] = np.asarray(res.results[c]["out"],
                                     dtype=np.float32).T
    return out


# revision 6
# speedup vs baseline: 1.1171x; 1.0109x over previous
"""Causal self-attention (B=4, N=2048, D=1024, single head) on 8 TRN2 NeuronCores.

Sharding: core c handles batch b = c//2, query shard h = c%2 with the
stride-2 interleave q_global = 2*j + h  (j = 0..1023).  The interleave makes
the causal-mask *tile structure* identical on every core (SPMD-uniform), so
fully-masked score tiles can be skipped structurally while the residual
diagonal masking is handled with per-core data (query-position tensor).

Per-core pipeline (all matmuls bf16 inputs, f32 PSUM accumulation):
  QT[e,n]  = WqT.T @ XTq   (+bq/32 folded into the PSUM->SBUF eviction)
  KT[e,k]  = WkT.T @ XT    (+bk in eviction)
  V[k,d]   = XT.T @ WvT    (+bv broadcast tile in eviction)
  ST[k,j]  = KT.T @ QT     (scores; 1/sqrt(D) folded into WqT host-side)
  E        = exp(ST) * causal_mask    (no max-subtraction: |scores| <~ 2)
  rowsum[j]= ones.T @ E    (PE reduction over k partitions)
  CT[d,j]  = V.T @ E
  OT[e,j]  = WoT.T @ CT
  out      = OT * (1/rowsum) + bo     (normalization deferred to the end)

Loops are ordered so each stationary (lhsT) operand feeds several
back-to-back matmuls, and PSUM evictions all run on the Vector engine.
No collectives: each core receives exactly the host-side shard it needs.
"""

import os
import numpy as np
import ml_dtypes

BF16 = ml_dtypes.bfloat16

N_CORES = 8
B, N, D = 4, 2048, 1024
NQ = 1024           # queries per core
P = 128             # partitions
ET = D // P         # 8  e-tiles
CT_ = D // P        # 8  contraction tiles of D
KT_ALL = N // P     # 16 key tiles
JCW = 512           # free-dim chunk
NJC = NQ // JCW     # 2

_cache = {}


def _build():
    from concourse import bacc, tile, mybir
    import concourse.bass as bass

    f32 = mybir.dt.float32
    bf16 = mybir.dt.bfloat16
    Exp = mybir.ActivationFunctionType.Exp
    is_ge = mybir.AluOpType.is_ge
    add = mybir.AluOpType.add
    mult = mybir.AluOpType.mult
    PSUM = bass.MemorySpace.PSUM

    nc = bacc.Bacc("TRN2", target_bir_lowering=False, debug=False,
                   num_devices=N_CORES)

    xt_d = nc.declare_dram_parameter("xt", [D, N], bf16, isOutput=False)
    xtq_d = nc.declare_dram_parameter("xtq", [D, NQ], bf16, isOutput=False)
    wqt_d = nc.declare_dram_parameter("wqt", [D, D], bf16, isOutput=False)
    wkt_d = nc.declare_dram_parameter("wkt", [D, D], bf16, isOutput=False)
    wvt_d = nc.declare_dram_parameter("wvt", [D, D], bf16, isOutput=False)
    wot_d = nc.declare_dram_parameter("wot", [D, D], bf16, isOutput=False)
    bqt_d = nc.declare_dram_parameter("bqt", [P, ET], f32, isOutput=False)
    bkt_d = nc.declare_dram_parameter("bkt", [P, ET], f32, isOutput=False)
    bbv_d = nc.declare_dram_parameter("bbv", [P, D], f32, isOutput=False)
    bot_d = nc.declare_dram_parameter("bot", [P, ET], f32, isOutput=False)
    bqp_d = nc.declare_dram_parameter("bqpos", [P, NQ], f32, isOutput=False)
    kpt_d = nc.declare_dram_parameter("kpost", [P, KT_ALL], f32, isOutput=False)
    out_d = nc.declare_dram_parameter("out", [D, NQ], f32, isOutput=True)

    with tile.TileContext(nc) as tc:
        with (
            tc.tile_pool(name="consts", bufs=1) as p_c,
            tc.tile_pool(name="w", bufs=10) as p_w,
            tc.tile_pool(name="qt", bufs=ET) as p_qt,
            tc.tile_pool(name="kt", bufs=ET) as p_kt,
            tc.tile_pool(name="v", bufs=KT_ALL) as p_v,
            tc.tile_pool(name="ps", bufs=5, space=PSUM) as p_ps,
            tc.tile_pool(name="rsps", bufs=2, space=PSUM) as p_rs,
        ):
            qt_tiles = [p_qt.tile([P, NQ], bf16, tag="qt", name="qt")
                        for _ in range(ET)]
            kt_tiles = [p_kt.tile([P, N], bf16, tag="kt", name="kt")
                        for _ in range(ET)]
            v_tiles = [p_v.tile([P, D], bf16, tag="v", name="v")
                       for _ in range(KT_ALL)]

            def load_w(dram):
                ts = []
                for ct in range(CT_):
                    t = p_w.tile([P, D], bf16, tag="w", name="w")
                    eng = nc.sync if ct % 2 == 0 else nc.scalar
                    eng.dma_start(t[:], dram[ct * P:(ct + 1) * P, :])
                    ts.append(t)
                return ts

            with (
                tc.tile_pool(name="xt", bufs=CT_) as p_xt,
                tc.tile_pool(name="xtq", bufs=CT_) as p_xtq,
            ):
                # ---- Q projection (DMAs interleaved: weight tile then X tile
                # so the PE can start on the first accumulation group ASAP) ----
                wq = []
                xtq_tiles = []
                for ct in range(CT_):
                    t = p_w.tile([P, D], bf16, tag="w", name="w")
                    eng = nc.sync if ct % 2 == 0 else nc.scalar
                    eng.dma_start(t[:], wqt_d[ct * P:(ct + 1) * P, :])
                    wq.append(t)
                    t2 = p_xtq.tile([P, NQ], bf16, tag="xtq", name="xtq")
                    eng2 = nc.gpsimd if ct % 2 == 0 else nc.sync
                    eng2.dma_start(t2[:], xtq_d[ct * P:(ct + 1) * P, :])
                    xtq_tiles.append(t2)
                bqt_t = p_c.tile([P, ET], f32, tag="bqt")
                nc.scalar.dma_start(bqt_t[:], bqt_d[:, :])

                for et in range(ET):
                    pss = [p_ps.tile([P, JCW], f32, tag="ps", name="ps")
                           for _ in range(NJC)]
                    for ct in range(CT_):
                        for jc in range(NJC):
                            nc.tensor.matmul(
                                pss[jc][:],
                                wq[ct][:, et * P:(et + 1) * P],
                                xtq_tiles[ct][:, jc * JCW:(jc + 1) * JCW],
                                start=(ct == 0), stop=(ct == CT_ - 1))
                    for jc in range(NJC):
                        nc.vector.tensor_scalar_add(
                            qt_tiles[et][:, jc * JCW:(jc + 1) * JCW],
                            pss[jc][:], bqt_t[:, et:et + 1])

                # ---- K projection ----
                wk = []
                xt_tiles = []
                for ct in range(CT_):
                    t = p_w.tile([P, D], bf16, tag="w", name="w")
                    eng = nc.sync if ct % 2 == 0 else nc.scalar
                    eng.dma_start(t[:], wkt_d[ct * P:(ct + 1) * P, :])
                    wk.append(t)
                    t2 = p_xt.tile([P, N], bf16, tag="xt", name="xt")
                    eng2 = nc.gpsimd if ct % 2 == 0 else nc.scalar
                    eng2.dma_start(t2[:], xt_d[ct * P:(ct + 1) * P, :])
                    xt_tiles.append(t2)
                bkt_t = p_c.tile([P, ET], f32, tag="bkt")
                nc.scalar.dma_start(bkt_t[:], bkt_d[:, :])

                for et in range(ET):
                    for kh in range(2):          # halves of the 4 k-chunks
                        pss = [p_ps.tile([P, JCW], f32, tag="ps", name="ps")
                               for _ in range(2)]
                        for ct in range(CT_):
                            for i, kc in enumerate((2 * kh, 2 * kh + 1)):
                                nc.tensor.matmul(
                                    pss[i][:],
                                    wk[ct][:, et * P:(et + 1) * P],
                                    xt_tiles[ct][:, kc * JCW:(kc + 1) * JCW],
                                    start=(ct == 0), stop=(ct == CT_ - 1))
                        for i, kc in enumerate((2 * kh, 2 * kh + 1)):
                            nc.vector.tensor_scalar_add(
                                kt_tiles[et][:, kc * JCW:(kc + 1) * JCW],
                                pss[i][:], bkt_t[:, et:et + 1])

                # ---- V projection ----
                wv = load_w(wvt_d)
                bbv_t = p_c.tile([P, D], f32, tag="bbv")
                nc.scalar.dma_start(bbv_t[:], bbv_d[:, :])
                for kt in range(KT_ALL):
                    pss = [p_ps.tile([P, JCW], f32, tag="ps", name="ps")
                           for _ in range(2)]
                    for ct in range(CT_):
                        for dc in range(2):
                            nc.tensor.matmul(
                                pss[dc][:],
                                xt_tiles[ct][:, kt * P:(kt + 1) * P],
                                wv[ct][:, dc * JCW:(dc + 1) * JCW],
                                start=(ct == 0), stop=(ct == CT_ - 1))
                    for dc in range(2):
                        nc.vector.tensor_tensor(
                            v_tiles[kt][:, dc * JCW:(dc + 1) * JCW],
                            pss[dc][:], bbv_t[:, dc * JCW:(dc + 1) * JCW], add)

            # Wo tiles + remaining consts
            wo = load_w(wot_d)
            ones_col = p_c.tile([P, 1], bf16, tag="ones_col")
            nc.gpsimd.memset(ones_col[:], 1.0)
            ones_col_f32 = p_c.tile([1, P], f32, tag="ones_col_f32")
            nc.gpsimd.memset(ones_col_f32[:], 1.0)
            bot_t = p_c.tile([P, ET], f32, tag="bot")
            nc.scalar.dma_start(bot_t[:], bot_d[:, :])
            bqpos_t = p_c.tile([P, NQ], f32, tag="bqpos")
            nc.scalar.dma_start(bqpos_t[:], bqp_d[:, :])
            kpost_t = p_c.tile([P, KT_ALL], f32, tag="kpost")
            nc.scalar.dma_start(kpost_t[:], kpt_d[:, :])

            with (
                tc.tile_pool(name="exp", bufs=KT_ALL + ET + 1) as p_exp,
                tc.tile_pool(name="raw", bufs=2) as p_raw,
                tc.tile_pool(name="ctx", bufs=2 * ET + 1) as p_ctx,
                tc.tile_pool(name="of", bufs=4) as p_of,
                tc.tile_pool(name="brec", bufs=2) as p_brec,
                tc.tile_pool(name="recip", bufs=2) as p_recip,
            ):
                # jc=0 covers global queries [0,1024): keys < 1024 (kt 0..7).
                # jc=1 covers [1024,2048): all 16 kt; kt 0..7 unmasked there.
                def jcs_of(kt):
                    return (0, 1) if kt < 8 else (1,)

                # ---- scores + exp + mask + rowsum ----
                rs_ps = {jc: p_rs.tile([1, JCW], f32, tag="rsps", name="rsps")
                         for jc in range(NJC)}
                exps = {}
                for kt in range(KT_ALL):
                    sts = {}
                    for jc in jcs_of(kt):
                        sts[jc] = p_ps.tile([P, JCW], f32, tag="ps", name="ps")
                    for et in range(ET):
                        for jc in jcs_of(kt):
                            nc.tensor.matmul(
                                sts[jc][:],
                                kt_tiles[et][:, kt * P:(kt + 1) * P],
                                qt_tiles[et][:, jc * JCW:(jc + 1) * JCW],
                                start=(et == 0), stop=(et == ET - 1))
                    for jc in jcs_of(kt):
                        ex = p_exp.tile([P, JCW], bf16, tag="exp", name="exp")
                        boundary = (kt >= 8 * jc)
                        if boundary:
                            raw = p_raw.tile([P, JCW], bf16, tag="raw",
                                             name="raw")
                            nc.scalar.activation(raw[:], sts[jc][:], Exp)
                            nc.vector.scalar_tensor_tensor(
                                ex[:],
                                bqpos_t[:, jc * JCW:(jc + 1) * JCW],
                                kpost_t[:, kt:kt + 1], raw[:],
                                is_ge, mult)
                        else:
                            nc.scalar.activation(ex[:], sts[jc][:], Exp)
                        exps[(jc, kt)] = ex
                        nkt = 8 if jc == 0 else 16
                        nc.tensor.matmul(rs_ps[jc][:], ones_col[:], ex[:],
                                         start=(kt == 0), stop=(kt == nkt - 1))

                # ---- 1/rowsum broadcast tiles ----
                brec = {}
                for jc in range(NJC):
                    recip_t = p_recip.tile([1, JCW], f32, tag="recip",
                                           name="recip")
                    nc.vector.reciprocal(recip_t[:], rs_ps[jc][:])
                    br_ps = p_ps.tile([P, JCW], f32, tag="ps", name="ps")
                    nc.tensor.matmul(br_ps[:], ones_col_f32[:], recip_t[:],
                                     start=True, stop=True)
                    bt = p_brec.tile([P, JCW], f32, tag="brec", name="brec")
                    nc.vector.tensor_copy(bt[:], br_ps[:])
                    brec[jc] = bt

                # ---- context ----
                ctxs = {}
                for dt in range(ET):
                    cps = {jc: p_ps.tile([P, JCW], f32, tag="ps", name="ps")
                           for jc in range(NJC)}
                    for kt in range(KT_ALL):
                        for jc in jcs_of(kt):
                            nkt = 8 if jc == 0 else 16
                            nc.tensor.matmul(
                                cps[jc][:],
                                v_tiles[kt][:, dt * P:(dt + 1) * P],
                                exps[(jc, kt)][:],
                                start=(kt == 0), stop=(kt == nkt - 1))
                    for jc in range(NJC):
                        ct_t = p_ctx.tile([P, JCW], bf16, tag="ctx",
                                          name="ctx")
                        if dt % 2 == 0 and dt < ET - 2:
                            nc.scalar.copy(ct_t[:], cps[jc][:])
                        else:
                            nc.vector.tensor_copy(ct_t[:], cps[jc][:])
                        ctxs[(jc, dt)] = ct_t

                # ---- output projection + normalize + bias ----
                for et in range(ET):
                    opss = {jc: p_ps.tile([P, JCW], f32, tag="ps", name="ps")
                            for jc in range(NJC)}
                    for dt in range(ET):
                        for jc in range(NJC):
                            nc.tensor.matmul(
                                opss[jc][:],
                                wo[dt][:, et * P:(et + 1) * P],
                                ctxs[(jc, dt)][:],
                                start=(dt == 0), stop=(dt == ET - 1))
                    for jc in range(NJC):
                        jsl = slice(jc * JCW, (jc + 1) * JCW)
                        of1 = p_of.tile([P, JCW], f32, tag="of", name="of")
                        nc.vector.tensor_mul(of1[:], opss[jc][:], brec[jc][:])
                        of2 = p_of.tile([P, JCW], f32, tag="of", name="of")
                        nc.vector.tensor_scalar_add(of2[:], of1[:],
                                                    bot_t[:, et:et + 1])
                        nc.sync.dma_start(out_d[et * P:(et + 1) * P, jsl],
                                          of2[:])

    nc.compile()
    return nc


def _prep_in_maps(X, Wq, bq, Wk, bk, Wv, bv, Wo, bo):
    s = np.float32(1.0 / np.sqrt(np.float32(D)))
    wqt = np.ascontiguousarray(Wq.T * s).astype(BF16)
    wkt = np.ascontiguousarray(Wk.T).astype(BF16)
    wvt = np.ascontiguousarray(Wv.T).astype(BF16)
    wot = np.ascontiguousarray(Wo.T).astype(BF16)
    bqt = np.ascontiguousarray((bq * s).reshape(ET, P).T).astype(np.float32)
    bkt = np.ascontiguousarray(bk.reshape(ET, P).T).astype(np.float32)
    bbv = np.ascontiguousarray(
        np.broadcast_to(bv[None, :], (P, D))).astype(np.float32)
    bot = np.ascontiguousarray(bo.reshape(ET, P).T).astype(np.float32)
    kpost = np.ascontiguousarray(
        np.arange(N, dtype=np.float32).reshape(KT_ALL, P).T)

    in_maps = []
    for c in range(N_CORES):
        b, h = c // 2, c % 2
        Xb = X[b]
        xt = np.ascontiguousarray(Xb.T).astype(BF16)
        xtq = np.ascontiguousarray(Xb[h::2].T).astype(BF16)
        qpos = (2.0 * np.arange(NQ, dtype=np.float32) + h)
        bqpos = np.ascontiguousarray(
            np.broadcast_to(qpos[None, :], (P, NQ))).astype(np.float32)
        in_maps.append({
            "xt": xt, "xtq": xtq,
            "wqt": wqt, "wkt": wkt, "wvt": wvt, "wot": wot,
            "bqt": bqt, "bkt": bkt, "bbv": bbv, "bot": bot,
            "bqpos": bqpos, "kpost": kpost,
        })
    return in_maps


last_exec_time_ns = None


def _ensure_ntff_hook():
    """Register the axon NTFF profile hook if the image's antenv lacks it."""
    try:
        from antenv.axon_hooks import get_axon_ntff_profile_hook  # noqa: F401
        return
    except ImportError:
        pass
    import sys
    import types
    mod = types.ModuleType("antenv.axon_hooks")
    mod._hook = None
    mod.set_axon_ntff_profile_hook = lambda h: setattr(mod, "_hook", h)
    mod.get_axon_ntff_profile_hook = lambda: mod._hook
    sys.modules["antenv.axon_hooks"] = mod
    try:
        import antenv
        antenv.axon_hooks = mod
    except ImportError:
        pass
    try:
        from trn_agent_boot.trn_boot import _ntff_profile_via_ctypes
        mod._hook = _ntff_profile_via_ctypes("/opt/axon/libaxon_pjrt.so")
    except Exception:
        pass


def kernel(X, Wq, bq, Wk, bk, Wv, bv, Wo, bo):
    global last_exec_time_ns
    from concourse.bass_utils import run_bass_kernel_spmd
    _ensure_ntff_hook()

    X = np.asarray(X, dtype=np.float32)
    args = [np.asarray(a, dtype=np.float32)
            for a in (Wq, bq, Wk, bk, Wv, bv, Wo, bo)]

    if "nc" not in _cache:
        _cache["nc"] = _build()
    nc = _cache["nc"]

    in_maps = _prep_in_maps(X, *args)
    kwargs = {}
    tmpdir = os.environ.get("KERNEL_TRACE_DIR")
    if tmpdir:
        kwargs = dict(trace=True, tmpdir=tmpdir)
    res = run_bass_kernel_spmd(nc, in_maps, core_ids=list(range(N_CORES)),
                               **kwargs)
    last_exec_time_ns = res.exec_time_ns

    out = np.empty((B, N, D), dtype=np.float32)
    for c in range(N_CORES):
        b, h = c // 2, c % 2
        out[b, h::2, :] = np.asarray(res.results[c]["out"],
                                     dtype=np.float32).T
    return out


# revision 7
# speedup vs baseline: 1.1831x; 1.0591x over previous
"""Causal self-attention (B=4, N=2048, D=1024, single head) on 8 TRN2 NeuronCores.

Sharding: core c handles batch b = c//2, query shard h = c%2 with the
stride-2 interleave q_global = 2*j + h  (j = 0..1023).  The interleave makes
the causal-mask *tile structure* identical on every core (SPMD-uniform), so
fully-masked score tiles can be skipped structurally while the residual
diagonal masking is handled with per-core data (query-position tensor).

Per-core pipeline (all matmuls bf16 inputs, f32 PSUM accumulation):
  QT[e,n]  = WqT.T @ XTq   (+bq/32 folded into the PSUM->SBUF eviction)
  KT[e,k]  = WkT.T @ XT    (+bk in eviction)
  V[k,d]   = XT.T @ WvT    (+bv broadcast tile in eviction)
  ST[k,j]  = KT.T @ QT     (scores; 1/sqrt(D) folded into WqT host-side)
  E        = exp(ST) * causal_mask    (no max-subtraction: |scores| <~ 2)
  rowsum[j]= ones.T @ E    (PE reduction over k partitions)
  CT[d,j]  = V.T @ E
  OT[e,j]  = WoT.T @ CT
  out      = OT * (1/rowsum) + bo     (normalization deferred to the end)

Loops are ordered so each stationary (lhsT) operand feeds several
back-to-back matmuls, and PSUM evictions all run on the Vector engine.
No collectives: each core receives exactly the host-side shard it needs.
"""

import os
import numpy as np
import ml_dtypes

BF16 = ml_dtypes.bfloat16

N_CORES = 8
B, N, D = 4, 2048, 1024
NQ = 1024           # queries per core
P = 128             # partitions
ET = D // P         # 8  e-tiles
CT_ = D // P        # 8  contraction tiles of D
KT_ALL = N // P     # 16 key tiles
JCW = 512           # free-dim chunk
NJC = NQ // JCW     # 2

_cache = {}


def _build():
    from concourse import bacc, tile, mybir
    import concourse.bass as bass

    f32 = mybir.dt.float32
    bf16 = mybir.dt.bfloat16
    Exp = mybir.ActivationFunctionType.Exp
    is_ge = mybir.AluOpType.is_ge
    add = mybir.AluOpType.add
    mult = mybir.AluOpType.mult
    PSUM = bass.MemorySpace.PSUM

    nc = bacc.Bacc("TRN2", target_bir_lowering=False, debug=False,
                   num_devices=N_CORES)

    xt_d = nc.declare_dram_parameter("xt", [D, N], bf16, isOutput=False)
    xtq_d = nc.declare_dram_parameter("xtq", [D, NQ], bf16, isOutput=False)
    wqt_d = nc.declare_dram_parameter("wqt", [D, D], bf16, isOutput=False)
    wkt_d = nc.declare_dram_parameter("wkt", [D, D], bf16, isOutput=False)
    wvt_d = nc.declare_dram_parameter("wvt", [D, D], bf16, isOutput=False)
    wot_d = nc.declare_dram_parameter("wot", [D, D], bf16, isOutput=False)
    bqt_d = nc.declare_dram_parameter("bqt", [P, ET], f32, isOutput=False)
    bkt_d = nc.declare_dram_parameter("bkt", [P, ET], f32, isOutput=False)
    bbv_d = nc.declare_dram_parameter("bbv", [P, D], f32, isOutput=False)
    bot_d = nc.declare_dram_parameter("bot", [P, ET], f32, isOutput=False)
    bqp_d = nc.declare_dram_parameter("bqpos", [P, NQ], f32, isOutput=False)
    kpt_d = nc.declare_dram_parameter("kpost", [P, KT_ALL], f32, isOutput=False)
    out_d = nc.declare_dram_parameter("out", [D, NQ], f32, isOutput=True)

    with tile.TileContext(nc) as tc:
        with (
            tc.tile_pool(name="consts", bufs=1) as p_c,
            tc.tile_pool(name="w", bufs=10) as p_w,
            tc.tile_pool(name="qt", bufs=ET) as p_qt,
            tc.tile_pool(name="kt", bufs=ET) as p_kt,
            tc.tile_pool(name="v", bufs=KT_ALL) as p_v,
            tc.tile_pool(name="ps", bufs=5, space=PSUM) as p_ps,
            tc.tile_pool(name="rsps", bufs=2, space=PSUM) as p_rs,
        ):
            qt_tiles = [p_qt.tile([P, NQ], bf16, tag="qt", name="qt")
                        for _ in range(ET)]
            kt_tiles = [p_kt.tile([P, N], bf16, tag="kt", name="kt")
                        for _ in range(ET)]
            v_tiles = [p_v.tile([P, D], bf16, tag="v", name="v")
                       for _ in range(KT_ALL)]

            def load_w(dram):
                ts = []
                for ct in range(CT_):
                    t = p_w.tile([P, D], bf16, tag="w", name="w")
                    eng = nc.sync if ct % 2 == 0 else nc.scalar
                    eng.dma_start(t[:], dram[ct * P:(ct + 1) * P, :])
                    ts.append(t)
                return ts

            with (
                tc.tile_pool(name="xt", bufs=CT_) as p_xt,
                tc.tile_pool(name="xtq", bufs=CT_) as p_xtq,
            ):
                # ---- Q projection (DMAs interleaved: weight tile then X tile
                # so the PE can start on the first accumulation group ASAP) ----
                wq = []
                xtq_tiles = []
                for ct in range(CT_):
                    t = p_w.tile([P, D], bf16, tag="w", name="w")
                    eng = nc.sync if ct % 2 == 0 else nc.scalar
                    eng.dma_start(t[:], wqt_d[ct * P:(ct + 1) * P, :])
                    wq.append(t)
                    t2 = p_xtq.tile([P, NQ], bf16, tag="xtq", name="xtq")
                    nc.gpsimd.dma_start(t2[:], xtq_d[ct * P:(ct + 1) * P, :])
                    xtq_tiles.append(t2)
                bqt_t = p_c.tile([P, ET], f32, tag="bqt")
                nc.scalar.dma_start(bqt_t[:], bqt_d[:, :])

                for et in range(ET):
                    pss = [p_ps.tile([P, JCW], f32, tag="ps", name="ps")
                           for _ in range(NJC)]
                    for ct in range(CT_):
                        for jc in range(NJC):
                            nc.tensor.matmul(
                                pss[jc][:],
                                wq[ct][:, et * P:(et + 1) * P],
                                xtq_tiles[ct][:, jc * JCW:(jc + 1) * JCW],
                                start=(ct == 0), stop=(ct == CT_ - 1))
                    for jc in range(NJC):
                        nc.vector.tensor_scalar_add(
                            qt_tiles[et][:, jc * JCW:(jc + 1) * JCW],
                            pss[jc][:], bqt_t[:, et:et + 1])

                # ---- K projection ----
                wk = []
                xt_tiles = []
                for ct in range(CT_):
                    t = p_w.tile([P, D], bf16, tag="w", name="w")
                    eng = nc.sync if ct % 2 == 0 else nc.scalar
                    eng.dma_start(t[:], wkt_d[ct * P:(ct + 1) * P, :])
                    wk.append(t)
                    t2 = p_xt.tile([P, N], bf16, tag="xt", name="xt")
                    eng2 = nc.gpsimd if ct % 2 == 0 else nc.scalar
                    eng2.dma_start(t2[:], xt_d[ct * P:(ct + 1) * P, :])
                    xt_tiles.append(t2)
                bkt_t = p_c.tile([P, ET], f32, tag="bkt")
                nc.scalar.dma_start(bkt_t[:], bkt_d[:, :])

                for et in range(ET):
                    for kh in range(2):          # halves of the 4 k-chunks
                        pss = [p_ps.tile([P, JCW], f32, tag="ps", name="ps")
                               for _ in range(2)]
                        for ct in range(CT_):
                            for i, kc in enumerate((2 * kh, 2 * kh + 1)):
                                nc.tensor.matmul(
                                    pss[i][:],
                                    wk[ct][:, et * P:(et + 1) * P],
                                    xt_tiles[ct][:, kc * JCW:(kc + 1) * JCW],
                                    start=(ct == 0), stop=(ct == CT_ - 1))
                        for i, kc in enumerate((2 * kh, 2 * kh + 1)):
                            nc.vector.tensor_scalar_add(
                                kt_tiles[et][:, kc * JCW:(kc + 1) * JCW],
                                pss[i][:], bkt_t[:, et:et + 1])

                # ---- V projection ----
                wv = load_w(wvt_d)
                bbv_t = p_c.tile([P, D], f32, tag="bbv")
                nc.scalar.dma_start(bbv_t[:], bbv_d[:, :])
                for kt in range(KT_ALL):
                    pss = [p_ps.tile([P, JCW], f32, tag="ps", name="ps")
                           for _ in range(2)]
                    for ct in range(CT_):
                        for dc in range(2):
                            nc.tensor.matmul(
                                pss[dc][:],
                                xt_tiles[ct][:, kt * P:(kt + 1) * P],
                                wv[ct][:, dc * JCW:(dc + 1) * JCW],
                                start=(ct == 0), stop=(ct == CT_ - 1))
                    for dc in range(2):
                        nc.vector.tensor_tensor(
                            v_tiles[kt][:, dc * JCW:(dc + 1) * JCW],
                            pss[dc][:], bbv_t[:, dc * JCW:(dc + 1) * JCW], add)

            # Wo tiles + remaining consts
            wo = load_w(wot_d)
            ones_col = p_c.tile([P, 1], bf16, tag="ones_col")
            nc.gpsimd.memset(ones_col[:], 1.0)
            ones_col_f32 = p_c.tile([1, P], f32, tag="ones_col_f32")
            nc.gpsimd.memset(ones_col_f32[:], 1.0)
            bot_t = p_c.tile([P, ET], f32, tag="bot")
            nc.scalar.dma_start(bot_t[:], bot_d[:, :])
            bqpos_t = p_c.tile([P, NQ], f32, tag="bqpos")
            nc.scalar.dma_start(bqpos_t[:], bqp_d[:, :])
            kpost_t = p_c.tile([P, KT_ALL], f32, tag="kpost")
            nc.scalar.dma_start(kpost_t[:], kpt_d[:, :])

            with (
                tc.tile_pool(name="exp", bufs=KT_ALL + ET + 1) as p_exp,
                tc.tile_pool(name="raw", bufs=2) as p_raw,
                tc.tile_pool(name="ctx", bufs=2 * ET + 1) as p_ctx,
                tc.tile_pool(name="of", bufs=4) as p_of,
                tc.tile_pool(name="brec", bufs=2) as p_brec,
                tc.tile_pool(name="recip", bufs=2) as p_recip,
            ):
                # jc=0 covers global queries [0,1024): keys < 1024 (kt 0..7).
                # jc=1 covers [1024,2048): all 16 kt; kt 0..7 unmasked there.
                def jcs_of(kt):
                    return (0, 1) if kt < 8 else (1,)

                # ---- scores + exp + mask + rowsum ----
                rs_ps = {jc: p_rs.tile([1, JCW], f32, tag="rsps", name="rsps")
                         for jc in range(NJC)}
                exps = {}
                for kt in range(KT_ALL):
                    sts = {}
                    for jc in jcs_of(kt):
                        sts[jc] = p_ps.tile([P, JCW], f32, tag="ps", name="ps")
                    for et in range(ET):
                        for jc in jcs_of(kt):
                            nc.tensor.matmul(
                                sts[jc][:],
                                kt_tiles[et][:, kt * P:(kt + 1) * P],
                                qt_tiles[et][:, jc * JCW:(jc + 1) * JCW],
                                start=(et == 0), stop=(et == ET - 1))
                    for jc in jcs_of(kt):
                        ex = p_exp.tile([P, JCW], bf16, tag="exp", name="exp")
                        boundary = (kt >= 8 * jc)
                        if boundary:
                            raw = p_raw.tile([P, JCW], bf16, tag="raw",
                                             name="raw")
                            nc.scalar.activation(raw[:], sts[jc][:], Exp)
                            nc.vector.scalar_tensor_tensor(
                                ex[:],
                                bqpos_t[:, jc * JCW:(jc + 1) * JCW],
                                kpost_t[:, kt:kt + 1], raw[:],
                                is_ge, mult)
                        else:
                            nc.scalar.activation(ex[:], sts[jc][:], Exp)
                        exps[(jc, kt)] = ex
                        nkt = 8 if jc == 0 else 16
                        nc.tensor.matmul(rs_ps[jc][:], ones_col[:], ex[:],
                                         start=(kt == 0), stop=(kt == nkt - 1))

                # ---- context ----
                ctxs = {}
                for dt in range(ET):
                    cps = {jc: p_ps.tile([P, JCW], f32, tag="ps", name="ps")
                           for jc in range(NJC)}
                    for kt in range(KT_ALL):
                        for jc in jcs_of(kt):
                            nkt = 8 if jc == 0 else 16
                            nc.tensor.matmul(
                                cps[jc][:],
                                v_tiles[kt][:, dt * P:(dt + 1) * P],
                                exps[(jc, kt)][:],
                                start=(kt == 0), stop=(kt == nkt - 1))
                    for jc in range(NJC):
                        ct_t = p_ctx.tile([P, JCW], bf16, tag="ctx",
                                          name="ctx")
                        if dt % 2 == 0 and dt < ET - 2:
                            nc.scalar.copy(ct_t[:], cps[jc][:])
                        else:
                            nc.vector.tensor_copy(ct_t[:], cps[jc][:])
                        ctxs[(jc, dt)] = ct_t

                # ---- 1/rowsum broadcast tiles (reciprocal overlaps ctx) ----
                brec = {}
                for jc in range(NJC):
                    recip_t = p_recip.tile([1, JCW], f32, tag="recip",
                                           name="recip")
                    nc.vector.reciprocal(recip_t[:], rs_ps[jc][:])
                    br_ps = p_ps.tile([P, JCW], f32, tag="ps", name="ps")
                    nc.tensor.matmul(br_ps[:], ones_col_f32[:], recip_t[:],
                                     start=True, stop=True)
                    bt = p_brec.tile([P, JCW], f32, tag="brec", name="brec")
                    nc.vector.tensor_copy(bt[:], br_ps[:])
                    brec[jc] = bt

                # ---- output projection + normalize + bias ----
                for et in range(ET):
                    opss = {jc: p_ps.tile([P, JCW], f32, tag="ps", name="ps")
                            for jc in range(NJC)}
                    for dt in range(ET):
                        for jc in range(NJC):
                            nc.tensor.matmul(
                                opss[jc][:],
                                wo[dt][:, et * P:(et + 1) * P],
                                ctxs[(jc, dt)][:],
                                start=(dt == 0), stop=(dt == ET - 1))
                    for jc in range(NJC):
                        jsl = slice(jc * JCW, (jc + 1) * JCW)
                        of1 = p_of.tile([P, JCW], f32, tag="of", name="of")
                        nc.vector.tensor_mul(of1[:], opss[jc][:], brec[jc][:])
                        of2 = p_of.tile([P, JCW], f32, tag="of", name="of")
                        nc.vector.tensor_scalar_add(of2[:], of1[:],
                                                    bot_t[:, et:et + 1])
                        nc.sync.dma_start(out_d[et * P:(et + 1) * P, jsl],
                                          of2[:])

    nc.compile()
    return nc


def _prep_in_maps(X, Wq, bq, Wk, bk, Wv, bv, Wo, bo):
    s = np.float32(1.0 / np.sqrt(np.float32(D)))
    wqt = np.ascontiguousarray(Wq.T * s).astype(BF16)
    wkt = np.ascontiguousarray(Wk.T).astype(BF16)
    wvt = np.ascontiguousarray(Wv.T).astype(BF16)
    wot = np.ascontiguousarray(Wo.T).astype(BF16)
    bqt = np.ascontiguousarray((bq * s).reshape(ET, P).T).astype(np.float32)
    bkt = np.ascontiguousarray(bk.reshape(ET, P).T).astype(np.float32)
    bbv = np.ascontiguousarray(
        np.broadcast_to(bv[None, :], (P, D))).astype(np.float32)
    bot = np.ascontiguousarray(bo.reshape(ET, P).T).astype(np.float32)
    kpost = np.ascontiguousarray(
        np.arange(N, dtype=np.float32).reshape(KT_ALL, P).T)

    in_maps = []
    for c in range(N_CORES):
        b, h = c // 2, c % 2
        Xb = X[b]
        xt = np.ascontiguousarray(Xb.T).astype(BF16)
        xtq = np.ascontiguousarray(Xb[h::2].T).astype(BF16)
        qpos = (2.0 * np.arange(NQ, dtype=np.float32) + h)
        bqpos = np.ascontiguousarray(
            np.broadcast_to(qpos[None, :], (P, NQ))).astype(np.float32)
        in_maps.append({
            "xt": xt, "xtq": xtq,
            "wqt": wqt, "wkt": wkt, "wvt": wvt, "wot": wot,
            "bqt": bqt, "bkt": bkt, "bbv": bbv, "bot": bot,
            "bqpos": bqpos, "kpost": kpost,
        })
    return in_maps


last_exec_time_ns = None


def _ensure_ntff_hook():
    """Register the axon NTFF profile hook if the image's antenv lacks it."""
    try:
        from antenv.axon_hooks import get_axon_ntff_profile_hook  # noqa: F401
        return
    except ImportError:
        pass
    import sys
    import types
    mod = types.ModuleType("antenv.axon_hooks")
    mod._hook = None
    mod.set_axon_ntff_profile_hook = lambda h: setattr(mod, "_hook", h)
    mod.get_axon_ntff_profile_hook = lambda: mod._hook
    sys.modules["antenv.axon_hooks"] = mod
    try:
        import antenv
        antenv.axon_hooks = mod
    except ImportError:
        pass
    try:
        from trn_agent_boot.trn_boot import _ntff_profile_via_ctypes
        mod._hook = _ntff_profile_via_ctypes("/opt/axon/libaxon_pjrt.so")
    except Exception:
        pass


def kernel(X, Wq, bq, Wk, bk, Wv, bv, Wo, bo):
    global last_exec_time_ns
    from concourse.bass_utils import run_bass_kernel_spmd
    _ensure_ntff_hook()

    X = np.asarray(X, dtype=np.float32)
    args = [np.asarray(a, dtype=np.float32)
            for a in (Wq, bq, Wk, bk, Wv, bv, Wo, bo)]

    if "nc" not in _cache:
        _cache["nc"] = _build()
    nc = _cache["nc"]

    in_maps = _prep_in_maps(X, *args)
    kwargs = {}
    tmpdir = os.environ.get("KERNEL_TRACE_DIR")
    if tmpdir:
        kwargs = dict(trace=True, tmpdir=tmpdir)
    res = run_bass_kernel_spmd(nc, in_maps, core_ids=list(range(N_CORES)),
                               **kwargs)
    last_exec_time_ns = res.exec_time_ns

    out = np.empty((B, N, D), dtype=np.float32)
    for c in range(N_CORES):
        b, h = c // 2, c % 2
        out[b, h::2, :] = np.asarray(res.results[c]["out"],
                                     dtype=np.float32).T
    return out
